# revision 1
# baseline (speedup 1.0000x reference)
"""4-layer transformer encoder (B=2, S=2048, D=1024, FF=4096, H=16) on 8 TRN2
NeuronCores.

Sharding: 4096 tokens split 512/core (cores 0-3 = batch 0, 4-7 = batch 1).
Weights replicated (host pre-tiles them so every weight DMA is contiguous).

v3 structure (vs the 2.68ms baseline):
 - The per-layer K/V AllGathers are fused and split by HEAD HALVES: two 1MB
   gathers (K chunks 0-3 + V cols for heads 0-7, then the other halves).
   AG-A is issued after only half the K/V projections, so its flight is
   covered by the remaining K/V/Q projections, and AG-B's flight is covered
   by attention on heads 0-7.  1MB payloads keep the one-hop Mesh algorithm.
 - The V payload is staged D-MAJOR per half (host-side weight-column
   permutation), so the gathered block DMAs into SBUF contiguously (1KB
   descriptors); head h's [128, 65] stationary block (64 V columns + the
   softmax-denominator ones column, memset locally) is a stride-8 AP.
 - Softmax denominators: one custom-DVE reciprocal_approx_fast per head
   (f32, ~51 ULP), rounded to f32r with one tensor_scalar, broadcast over DH
   partitions with a ones-matmul; normalization multiplies ctx straight out
   of PSUM.  No DRAM round-trips.
 - LN rstd = exp(-0.5*ln(var+eps)) so every ACT call stays in the
   natural_log_exp table set (no ~2.7us table swaps per layer).
 - Residual stream and LN statistics stay fp32(r) (precision); only GEMM
   inputs are cast to bf16.

Activations live transposed in SBUF (x^T: [D partitions, tokens free]) so no
on-device transposes are needed anywhere; the host transposes the input shard
once and the output shard back.
"""
import sys
if '/opt/trn_rl_repo' not in sys.path:
    sys.path.insert(0, '/opt/trn_rl_repo')

import numpy as np
import ml_dtypes

import concourse.bass as bass
import concourse.mybir as mybir
import concourse.tile as tile
import concourse.bacc as bacc
from concourse import bass_utils

# problem config (hardcoded per contest rules)
L = 4
D = 1024
FF = 4096
H = 16
DH = 64
B = 2
S = 2048
EPS = 1e-6
SCALE = 1.0 / 8.0  # 1/sqrt(DH)

NCORES = 8
TOK = 512           # tokens per core
P = 128
DC = D // P         # 8 d-chunks
FC = FF // P        # 32 ff-chunks
NK = S // P         # 16 k-token chunks
R = 4               # ranks per quad (cores sharing one batch element)
RGROUPS = [[0, 1, 2, 3], [4, 5, 6, 7]]
HA = DH + 1         # V head block augmented with a ones column
HH = H // 2         # heads per gather half

dt = mybir.dt
AF = mybir.ActivationFunctionType
OP = mybir.AluOpType


def build(n_layers=L):
    nc = bacc.Bacc("TRN2", target_bir_lowering=False, debug=False,
                   num_devices=NCORES)
    f32, f32r, bf16 = dt.float32, dt.float32r, dt.bfloat16

    xT_d = nc.dram_tensor("xT", [D, TOK], f32, kind="ExternalInput")
    # weights pre-tiled on host: [n_layers, G, P, DC, 256]
    wq_d = nc.dram_tensor("wq", [n_layers, 4, P, DC, 256], bf16,
                          kind="ExternalInput")
    wk_d = nc.dram_tensor("wk", [n_layers, 4, P, DC, 256], bf16,
                          kind="ExternalInput")
    wv_d = nc.dram_tensor("wv", [n_layers, 4, P, DC, 256], bf16,
                          kind="ExternalInput")
    wp_d = nc.dram_tensor("wp", [n_layers, 4, P, DC, 256], bf16,
                          kind="ExternalInput")
    w1_d = nc.dram_tensor("w1", [n_layers, 16, P, DC, 256], bf16,
                          kind="ExternalInput")
    w2_d = nc.dram_tensor("w2", [n_layers, 16, P, DC, 256], bf16,
                          kind="ExternalInput")
    bq_d = nc.dram_tensor("bq", [n_layers, D], f32, kind="ExternalInput")
    bk_d = nc.dram_tensor("bk", [n_layers, D], f32, kind="ExternalInput")
    bp_d = nc.dram_tensor("bp", [n_layers, D], f32, kind="ExternalInput")
    b1_d = nc.dram_tensor("b1", [n_layers, FF], f32, kind="ExternalInput")
    b2_d = nc.dram_tensor("b2", [n_layers, D], f32, kind="ExternalInput")
    g1_d = nc.dram_tensor("g1", [n_layers, D], f32, kind="ExternalInput")
    be1_d = nc.dram_tensor("be1", [n_layers, D], f32, kind="ExternalInput")
    g2_d = nc.dram_tensor("g2", [n_layers, D], f32, kind="ExternalInput")
    be2_d = nc.dram_tensor("be2", [n_layers, D], f32, kind="ExternalInput")
    out_d = nc.dram_tensor("outT", [D, TOK], f32, kind="ExternalOutput")

    with tile.TileContext(nc) as tc:
        with (
            tc.tile_pool(name="pers", bufs=1) as pers,
            tc.tile_pool(name="sb", bufs=1) as sb,
            tc.tile_pool(name="ps", bufs=1, space="PSUM") as ps,
            tc.tile_pool(name="dram", bufs=1, space="DRAM") as dram,
        ):
            ones_f = pers.tile([P, P], f32)
            nc.vector.memset(ones_f[:], 1.0)
            ones = pers.tile([P, P], f32r)
            nc.vector.tensor_copy(out=ones[:], in_=ones_f[:])
            eps_sb = pers.tile([1, 1], f32)
            nc.vector.memset(eps_sb[:], EPS)

            def load_param(name, src, nchunk):
                t = pers.tile([P, n_layers, nchunk], f32, name=name)
                nc.sync.dma_start(
                    t[:], src[:, :].rearrange("l (c p) -> p l c", p=P))
                return t

            bq_sb = load_param("bq_sb", bq_d, DC)
            bk_sb = load_param("bk_sb", bk_d, DC)
            bp_sb = load_param("bp_sb", bp_d, DC)
            b2_sb = load_param("b2_sb", b2_d, DC)
            g1_sb = load_param("g1_sb", g1_d, DC)
            be1_sb = load_param("be1_sb", be1_d, DC)
            g2_sb = load_param("g2_sb", g2_d, DC)
            be2_sb = load_param("be2_sb", be2_d, DC)
            b1_sb = load_param("b1_sb", b1_d, FC)

            xT = sb.tile([P, DC, TOK], f32r, tag="xT", bufs=2, name="xT0")
            nc.sync.dma_start(
                xT[:],
                xT_d[:, :].rearrange("(c p) t -> p c t", p=P).bitcast(f32r))

            def cast_bf16(xsrc, name):
                xb = sb.tile([P, DC, TOK], bf16, tag="xTb", bufs=2, name=name)
                for c in range(DC):
                    nc.vector.tensor_copy(out=xb[:, c, :],
                                          in_=xsrc[:, c, :].bitcast(f32))
                return xb
            xTb = cast_bf16(xT, "xTb0")

            def wtile(w_d, l, g, name):
                t = sb.tile([P, DC, 256], bf16, tag="wblk", bufs=3, name=name)
                nc.sync.dma_start(t[:], w_d[l, g])
                return t

            def round_f32r(dst_tag, src, name):
                """f32 -> f32r rounding hop so the PE broadcast matmul input
                satisfies the f32r-rounding rule."""
                t = sb.tile(list(src.shape), dt.float32r, tag=dst_tag, bufs=4,
                            name=name)
                nc.vector.tensor_scalar_mul(t[:], src[:], 1.0)
                return t

            def layernorm(l, t1, g_sb, be_sb, xout):
                """xout[:, c, :] = LN(t1) over the partition (d) axis."""
                psum_s = ps.tile([1, TOK], f32, tag="mm", bufs=2,
                                 name="psum_s")
                psum_sq = ps.tile([1, TOK], f32, tag="mm", bufs=2,
                                  name="psum_sq")
                for c in range(DC):
                    nc.tensor.matmul(psum_s[:], ones[:, 0:1], t1[:, c, :],
                                     start=(c == 0), stop=(c == DC - 1))
                for c in range(DC):
                    sqc = sb.tile([P, TOK], f32r, tag="sq", bufs=2, name="sqc")
                    nc.scalar.square(sqc[:], t1[:, c, :])
                    nc.tensor.matmul(psum_sq[:], ones[:, 0:1], sqc[:],
                                     start=(c == 0), stop=(c == DC - 1))
                mean = sb.tile([1, TOK], f32r, tag="vec", bufs=4, name="mean")
                nc.vector.tensor_scalar_mul(mean[:], psum_s[:], 1.0 / D)
                ms = sb.tile([1, TOK], f32, tag="vec", bufs=4, name="ms")
                nc.vector.tensor_scalar_mul(ms[:], psum_sq[:], 1.0 / D)
                var = sb.tile([1, TOK], f32, tag="vec", bufs=4, name="var")
                # var = ms - mean*mean = (mean * -mean) * mean + ms
                nc.vector.scalar_tensor_tensor(
                    out=var[:], in0=mean[:].bitcast(f32), scalar=-1.0,
                    in1=mean[:].bitcast(f32), op0=OP.mult, op1=OP.mult)
                nc.vector.tensor_sub(var[:], ms[:], var[:])
                # rstd = var^-0.5 via ln+exp: keeps every ACT call in the
                # natural_log_exp table set (no ~2.7us per-layer set swaps)
                lnv = sb.tile([1, TOK], f32, tag="vec", bufs=4, name="lnv")
                nc.scalar.activation(lnv[:], var[:], AF.Ln, bias=eps_sb[:])
                rstd = sb.tile([1, TOK], f32, tag="vec", bufs=4, name="rstd")
                nc.scalar.activation(rstd[:], lnv[:], AF.Exp, scale=-0.5)
                rstd_r = round_f32r("vec", rstd[:], "rstd_r")
                pm = ps.tile([P, TOK], f32, tag="mm", bufs=2, name="pm")
                nc.tensor.matmul(pm[:], ones[0:1, :], mean[:],
                                 start=True, stop=True)
                mrep = sb.tile([P, TOK], f32, tag="mrep", bufs=1, name="mrep")
                nc.scalar.copy(mrep[:], pm[:])
                pr = ps.tile([P, TOK], f32, tag="mm", bufs=2, name="pr")
                nc.tensor.matmul(pr[:], ones[0:1, :], rstd_r[:],
                                 start=True, stop=True)
                rrep = sb.tile([P, TOK], f32, tag="rrepLN", bufs=1,
                               name="rrep")
                nc.scalar.copy(rrep[:], pr[:])
                for c in range(DC):
                    d1 = sb.tile([P, TOK], f32, tag="lnscr", bufs=2,
                                 name="d1")
                    nc.vector.tensor_sub(d1[:], t1[:, c, :].bitcast(f32),
                                         mrep[:])
                    d2 = sb.tile([P, TOK], f32, tag="lnscr", bufs=2,
                                 name="d2")
                    nc.vector.tensor_mul(d2[:], d1[:], rrep[:])
                    nc.vector.tensor_scalar(
                        out=xout[:, c, :], in0=d2[:],
                        scalar1=g_sb[:, l, c:c + 1],
                        scalar2=be_sb[:, l, c:c + 1],
                        op0=OP.mult, op1=OP.add)

            def kv_half(l, half, cc_t, wts, gate=None):
                """K chunks and d-major V cols for heads half*8..half*8+8,
                staged into cc_t[0]=K [4, P, TOK], cc_t[1]=V [4tc, P, 512].
                wts = [wk(g0), wv(g0), wk(g1), wv(g1)] preloaded tiles.
                gate: optional all-zero [P, 256] tile whose producers must
                complete before the LAST staged block (and therefore the
                AllGather of this half) can go out."""
                for gl in range(2):
                    g = 2 * half + gl
                    # K projection for out-chunks 2g, 2g+1
                    wt = wts[2 * gl]
                    pks = [ps.tile([P, TOK], f32, tag="mm", bufs=2,
                                   name=f"pk{i}") for i in range(2)]
                    for kc in range(DC):
                        for cc in range(2):
                            nc.tensor.matmul(
                                pks[cc][:],
                                wt[:, kc, 128 * cc:128 * (cc + 1)],
                                xTb[:, kc, :],
                                start=(kc == 0), stop=(kc == DC - 1))
                    for cc in range(2):
                        c = 2 * g + cc
                        kst = sb.tile([P, TOK], bf16, tag="kvstage", bufs=3,
                                      name="kst")
                        nc.scalar.activation(kst[:], pks[cc][:], AF.Identity,
                                             bias=bk_sb[:, l, c:c + 1])
                        nc.sync.dma_start(cc_t[0, c - 4 * half], kst[:])
                    # V projection (token-major, d-major cols per half)
                    wtv = wts[2 * gl + 1]
                    for tp_ in range(2):
                        pvs = [ps.tile([P, 256], f32, tag="mm", bufs=2,
                                       name=f"pv{i}") for i in range(2)]
                        for kc in range(DC):
                            for i in range(2):
                                tcc = 2 * tp_ + i
                                nc.tensor.matmul(
                                    pvs[i][:],
                                    xTb[:, kc, 128 * tcc:128 * (tcc + 1)],
                                    wtv[:, kc, :],
                                    start=(kc == 0), stop=(kc == DC - 1))
                        for i in range(2):
                            tcc = 2 * tp_ + i
                            vst = sb.tile([P, 256], bf16, tag="kvstage",
                                          bufs=3, name="vst")
                            if gate is not None and gl == 1 and tp_ == 1 \
                                    and i == 1:
                                nc.vector.tensor_add(vst[:], pvs[i][:],
                                                     gate[:])
                            else:
                                nc.vector.tensor_copy(out=vst[:],
                                                      in_=pvs[i][:])
                            nc.sync.dma_start(
                                cc_t[1, tcc, :, 256 * gl:256 * (gl + 1)],
                                vst[:])

            def load_gathered(cco, name):
                """[R, 2, 4, P, 512] gathered d-major V -> v_sb
                [P, R, 4, 520]: cols d*8+h for d<64, plus a ones block at
                cols 512..519 (so a stride-8 AP reads head h's [65] block
                with the softmax-denominator ones column as d=64)."""
                v_sb = sb.tile([P, R, 4, HH * HA], bf16, tag=f"Vg{name}",
                               bufs=1, name=f"v_sb{name}")
                nc.vector.memset(v_sb[:, :, :, HH * DH:], 1.0)
                for r_ in range(R):
                    nc.sync.dma_start(
                        v_sb[:, r_, :, 0:HH * DH],
                        cco[r_, 1].rearrange("t p q -> p t q"))
                return v_sb

            def load_ktc(cco, cl):
                ktc = sb.tile([P, R, TOK], bf16, tag="KTc", bufs=4,
                              name="ktc")
                nc.sync.dma_start(
                    ktc[:], cco[:, 0, cl].rearrange("r p t -> p r t"))
                return ktc

            def kv_wts(l, half):
                return [wtile(wk_d, l, 2 * half, "wkt"),
                        wtile(wv_d, l, 2 * half, "wvt"),
                        wtile(wk_d, l, 2 * half + 1, "wkt"),
                        wtile(wv_d, l, 2 * half + 1, "wvt")]

            for l in range(n_layers):
                # ---- K/V projections, head-half A, staged + gathered ------
                ccA = dram.tile([2, 4, P, TOK], bf16, tag="ccA", bufs=2,
                                name="ccA")
                kv_half(l, 0, ccA, kv_wts(l, 0))
                ccAo = dram.tile([R, 2, 4, P, TOK], bf16, tag="ccAo", bufs=2,
                                 name="ccAo")
                nc.gpsimd.collective_compute(
                    "AllGather", OP.bypass, replica_groups=RGROUPS,
                    ins=[ccA[:].opt()], outs=[ccAo[:].opt()])
                # B-half and Q weight loads go out BEFORE the gated A-reads
                # so the cover compute isn't starved behind AG-A.
                wtsB = kv_wts(l, 1)
                wqts = [wtile(wq_d, l, g, "wqt") for g in range(4)]
                # issue the half-A reads NOW (ahead of the ccB staging DMAs
                # in the sync FIFO): they unblock the moment AG-A lands and
                # their transfers beat AG-B's ring traffic.
                v_sbA = load_gathered(ccAo, "A")
                ktcs = {c: load_ktc(ccAo, c) for c in range(4)}
                # all-zero gate computed FROM the half-A loads: the last
                # ccB staging block adds it, so AG-B's ring traffic (which
                # stalls in-flight reads of gathered data on the shared DMA
                # engines) cannot start until attention's half-A inputs are
                # safely in SBUF.
                z_a = sb.tile([P, 256], bf16, tag="zg", bufs=2, name="z_a")
                nc.vector.tensor_scalar_mul(z_a[:], v_sbA[:, 3, 3, 0:256],
                                            0.0)
                zg = sb.tile([P, 256], bf16, tag="zg", bufs=2, name="zg")
                nc.vector.scalar_tensor_tensor(
                    out=zg[:], in0=ktcs[3][:, 3, 0:256], scalar=0.0,
                    in1=z_a[:], op0=OP.mult, op1=OP.add)

                # ---- head-half B ------------------------------------------
                ccB = dram.tile([2, 4, P, TOK], bf16, tag="ccB", bufs=2,
                                name="ccB")
                kv_half(l, 1, ccB, wtsB, gate=zg)
                ccBo = dram.tile([R, 2, 4, P, TOK], bf16, tag="ccBo", bufs=2,
                                 name="ccBo")
                nc.gpsimd.collective_compute(
                    "AllGather", OP.bypass, replica_groups=RGROUPS,
                    ins=[ccB[:].opt()], outs=[ccBo[:].opt()])

                # ---- Q projection (stays local, bf16) ---------------------
                QT = sb.tile([P, DC, TOK], bf16, tag="QT", bufs=1, name="QT")
                for g in range(4):
                    wt = wqts[g]
                    pqs = [ps.tile([P, TOK], f32, tag="mm", bufs=2,
                                   name=f"pq{i}") for i in range(2)]
                    for kc in range(DC):
                        for cc in range(2):
                            nc.tensor.matmul(
                                pqs[cc][:],
                                wt[:, kc, 128 * cc:128 * (cc + 1)],
                                xTb[:, kc, :],
                                start=(kc == 0), stop=(kc == DC - 1))
                    for cc in range(2):
                        c = 2 * g + cc
                        nc.scalar.activation(QT[:, c, :], pqs[cc][:],
                                             AF.Identity,
                                             bias=bq_sb[:, l, c:c + 1])

                # ---- attention: head pair per c, halves pipelined ---------
                ctxT = sb.tile([P, DC, TOK], bf16, tag="ctxTb", bufs=1,
                               name="ctxT")
                v_sbs = [v_sbA, None]
                for c in range(DC):
                    half = c // 4
                    if c == 1:
                        # B-half reads: issued early in the sync FIFO, they
                        # wait on AG-B and land during attention on half A.
                        v_sbs[1] = load_gathered(ccBo, "B")
                        for cb in range(4, DC):
                            ktcs[cb] = load_ktc(ccBo, cb - 4)
                    v_sb = v_sbs[half]
                    ktc = ktcs[c]
                    pc0 = ps.tile([HA, TOK], f32, tag="ctx", bufs=2,
                                  name="pc0")
                    pc1 = ps.tile([HA, TOK], f32, tag="ctx", bufs=2,
                                  name="pc1")
                    for kp in range(NK // 2):
                        sc0 = ps.tile([P, 2, TOK], f32, tag="sc2", bufs=2,
                                      name="sc0")
                        sc1 = ps.tile([P, 2, TOK], f32, tag="sc2", bufs=2,
                                      name="sc1")
                        for i in range(2):
                            kc = 2 * kp + i
                            r_, j = divmod(kc, 4)
                            nc.tensor.matmul(
                                sc0[:, i, :],
                                ktc[0:DH, r_, 128 * j:128 * (j + 1)],
                                QT[0:DH, c, :], start=True, stop=True,
                                tile_position=(0, 0))
                            nc.tensor.matmul(
                                sc1[:, i, :],
                                ktc[DH:P, r_, 128 * j:128 * (j + 1)],
                                QT[DH:P, c, :], start=True, stop=True,
                                tile_position=(64, 0))
                        e0 = sb.tile([P, 2, TOK], bf16, tag="E", bufs=4,
                                     name="e0")
                        nc.scalar.activation(e0[:], sc0[:], AF.Exp,
                                             scale=SCALE)
                        e1 = sb.tile([P, 2, TOK], bf16, tag="E", bufs=4,
                                     name="e1")
                        nc.scalar.activation(e1[:], sc1[:], AF.Exp,
                                             scale=SCALE)
                        for i in range(2):
                            kc = 2 * kp + i
                            r_, j = divmod(kc, 4)
                            h0 = 2 * c - 8 * half
                            vv = v_sb[:, r_, j].rearrange(
                                "p (d h) -> p h d", h=HH)
                            nc.tensor.matmul(
                                pc0[:], vv[:, h0, :],
                                e0[:, i, :], start=(kc == 0),
                                stop=(kc == NK - 1))
                            nc.tensor.matmul(
                                pc1[:], vv[:, h0 + 1, :],
                                e1[:, i, :], start=(kc == 0),
                                stop=(kc == NK - 1))
                    # softmax denominators: fast-reciprocal the ones-column
                    # accumulator row, round to f32r, broadcast over DH
                    # partitions with a ones-matmul, normalize out of PSUM.
                    for h, pch in ((0, pc0), (1, pc1)):
                        # partition-0 tiles: the broadcast matmul's 1-row
                        # operands must start at partition 0 (no
                        # tile_position), like the baseline's rrow form.
                        # Custom-DVE ops read from SBUF (PSUM-source custom
                        # uops are unvalidated on HW) - evacuate first.
                        den = sb.tile([1, TOK], f32, tag="den", bufs=2,
                                      name="den")
                        nc.vector.tensor_copy(out=den[:], in_=pch[DH:HA, :])
                        rcp = sb.tile([1, TOK], f32, tag="rcp", bufs=2,
                                      name="rcp")
                        nc.vector.reciprocal_approx_fast(
                            rcp[:], den[:])
                        rcp_r = sb.tile([1, TOK], f32r, tag="rcpr", bufs=2,
                                        name="rcp_r")
                        nc.vector.tensor_scalar_mul(rcp_r[:], rcp[:], 1.0)
                        prr = ps.tile([DH, TOK], f32, tag="mm", bufs=2,
                                      name="prr")
                        nc.tensor.matmul(
                            prr[:], ones[0:1, 0:DH], rcp_r[:],
                            start=True, stop=True)
                        rr = sb.tile([DH, TOK], f32, tag="rrep", bufs=2,
                                     name="rr")
                        nc.vector.tensor_copy(out=rr[:], in_=prr[:])
                        nc.vector.tensor_mul(
                            ctxT[DH * h:DH * (h + 1), c, :],
                            pch[0:DH, :], rr[:])

                # ---- output projection + residual -------------------------
                t1a = sb.tile([P, DC, TOK], f32r, tag="big2", bufs=1,
                              name="t1a")
                for g in range(4):
                    wt = wtile(wp_d, l, g, "wpt")
                    pps = [ps.tile([P, TOK], f32, tag="mm", bufs=2,
                                   name=f"pp{i}") for i in range(2)]
                    for kc in range(DC):
                        for cc in range(2):
                            nc.tensor.matmul(
                                pps[cc][:],
                                wt[:, kc, 128 * cc:128 * (cc + 1)],
                                ctxT[:, kc, :],
                                start=(kc == 0), stop=(kc == DC - 1))
                    for cc in range(2):
                        c = 2 * g + cc
                        nc.vector.scalar_tensor_tensor(
                            out=t1a[:, c, :], in0=pps[cc][:],
                            scalar=bp_sb[:, l, c:c + 1],
                            in1=xT[:, c, :].bitcast(f32),
                            op0=OP.add, op1=OP.add)

                xmid = sb.tile([P, DC, TOK], f32r, tag="xT", bufs=2,
                               name="xmid")
                layernorm(l, t1a, g1_sb, be1_sb, xmid)
                xmidb = cast_bf16(xmid, "xmidb")

                # ---- FFN --------------------------------------------------
                t1f = sb.tile([P, DC, TOK], f32r, tag="big2", bufs=1,
                              name="t1f")
                for q in range(4):
                    hT = sb.tile([P, DC, TOK], bf16, tag="hT", bufs=2,
                                 name="hT")
                    for g in range(4):
                        wt = wtile(w1_d, l, 4 * q + g, "w1t")
                        phs = [ps.tile([P, TOK], f32, tag="mm", bufs=2,
                                       name=f"ph{i}") for i in range(2)]
                        for kc in range(DC):
                            for cc in range(2):
                                nc.tensor.matmul(
                                    phs[cc][:],
                                    wt[:, kc, 128 * cc:128 * (cc + 1)],
                                    xmidb[:, kc, :],
                                    start=(kc == 0), stop=(kc == DC - 1))
                        for cc in range(2):
                            fcl = 2 * g + cc
                            fcg = q * DC + fcl
                            nc.scalar.activation(
                                hT[:, fcl, :], phs[cc][:], AF.Relu,
                                bias=b1_sb[:, l, fcg:fcg + 1])
                    for gc in range(4):
                        w2t = wtile(w2_d, l, 4 * q + gc, "w2t")
                        pys = [ps.tile([P, TOK], f32, tag="mm", bufs=2,
                                       name=f"py{i}") for i in range(2)]
                        for kc in range(DC):
                            for cc in range(2):
                                nc.tensor.matmul(
                                    pys[cc][:],
                                    w2t[:, kc, 128 * cc:128 * (cc + 1)],
                                    hT[:, kc, :],
                                    start=(kc == 0), stop=(kc == DC - 1))
                        for cc in range(2):
                            c = 2 * gc + cc
                            py = pys[cc]
                            if q == 0:
                                nc.vector.scalar_tensor_tensor(
                                    out=t1f[:, c, :], in0=py[:],
                                    scalar=b2_sb[:, l, c:c + 1],
                                    in1=xmid[:, c, :].bitcast(f32),
                                    op0=OP.add, op1=OP.add)
                            else:
                                nc.vector.tensor_add(
                                    t1f[:, c, :], py[:],
                                    t1f[:, c, :].bitcast(f32))

                xnext = sb.tile([P, DC, TOK], f32r, tag="xT", bufs=2,
                                name="xnext")
                layernorm(l, t1f, g2_sb, be2_sb, xnext)
                xT = xnext
                xTb = cast_bf16(xT, "xTbn")

            nc.sync.dma_start(
                out_d[:, :].rearrange("(c p) t -> p c t", p=P).bitcast(f32r),
                xT[:])
    nc.finalize()
    return nc


_NC_CACHE = {}


def get_nc(n_layers=L):
    if n_layers not in _NC_CACHE:
        _NC_CACHE[n_layers] = build(n_layers)
    return _NC_CACHE[n_layers]


def _tile_weight(w, G):
    """[L, K, O] -> [L, G_total, P, K//128, 256] matching wtile() blocks.

    For K=D (projections): block g covers out-cols 256g..256g+256, all K.
    For W1/W2 the same formula applies per 1024-col quarter group because
    blocks are indexed 4q+g and cover kc-chunks of the full K dim for W1,
    and kc-local chunks for W2 (handled by the caller's slicing)."""
    Lw, K, O = w.shape
    t = w.reshape(Lw, K // P, P, O // 256, 256).transpose(0, 3, 2, 1, 4)
    return np.ascontiguousarray(t)


def _tile_wv(wv):
    """[L, K, O] -> [L, 4, P, K//128, 256] with a PER-HALF d-major column
    permutation so the staged V payload is directly the v_sb layout:
    block g (half=g//2, gl=g%2), psum col j -> payload col q=256*gl+j,
    d = q//8, h = q%8, original column 64*(8*half+h)+d."""
    Lw, K, O = wv.shape
    w5 = wv.reshape(Lw, K // P, P, O)
    t = np.empty((Lw, 4, P, K // P, 256), wv.dtype)
    for g in range(4):
        half, gl = divmod(g, 2)
        q = 256 * gl + np.arange(256)
        o = 64 * (8 * half + q % 8) + q // 8
        t[:, g] = w5[:, :, :, o].transpose(0, 2, 1, 3)
    return np.ascontiguousarray(t)


def _tile_w2(w2):
    """[L, FF, D] -> [L, 16, P, 8, 256]; block 4q+gc covers W2 rows
    1024q..1024(q+1), cols 256gc..256(gc+1)."""
    Lw = w2.shape[0]
    t = w2.reshape(Lw, 4, 8, P, 4, 256).transpose(0, 1, 4, 3, 2, 5)
    return np.ascontiguousarray(t.reshape(Lw, 16, P, 8, 256))


def run(inputs, n_layers=L, trace=False):
    """inputs: the full setup_inputs() dict. Returns (out, BassKernelResults)."""
    hs = np.asarray(inputs["hidden_states"], np.float32)
    f = lambda k: np.ascontiguousarray(np.asarray(inputs[k], np.float32))
    Wq, Wk, Wv, Wp = f("Wq"), f("Wk"), f("Wv"), f("Wp")
    W1, W2 = f("W1"), f("W2")
    bq, bk, bv, bp = f("bq"), f("bk"), f("bv"), f("bp")
    b1, b2 = f("b1"), f("b2")
    g1, be1, g2, be2 = f("ln1_g"), f("ln1_b"), f("ln2_g"), f("ln2_b")
    # fold the V bias through the output projection: P(V + 1 bv^T) Wp + bp
    # = P V Wp + r*(bv Wp) + bp, and after normalization r/r = 1.
    bp_eff = (bp + np.einsum("ld,ldo->lo", bv, Wp)).astype(np.float32)

    bf = ml_dtypes.bfloat16
    wq_t = _tile_weight(Wq[:n_layers], 4).astype(bf)
    wk_t = _tile_weight(Wk[:n_layers], 4).astype(bf)
    wv_t = _tile_wv(Wv[:n_layers]).astype(bf)
    wp_t = _tile_weight(Wp[:n_layers], 4).astype(bf)
    w1_t = _tile_weight(W1[:n_layers], 16).astype(bf)
    w2_t = _tile_w2(W2[:n_layers]).astype(bf)

    xflat = hs.reshape(B * S, D)
    in_maps = []
    for i in range(NCORES):
        xTi = np.ascontiguousarray(xflat[i * TOK:(i + 1) * TOK].T)
        in_maps.append(dict(
            xT=xTi,
            wq=wq_t, wk=wk_t, wv=wv_t, wp=wp_t, w1=w1_t, w2=w2_t,
            bq=bq[:n_layers], bk=bk[:n_layers], bp=bp_eff[:n_layers],
            b1=b1[:n_layers], b2=b2[:n_layers], g1=g1[:n_layers],
            be1=be1[:n_layers], g2=g2[:n_layers], be2=be2[:n_layers]))
    nc = get_nc(n_layers)
    res = bass_utils.run_bass_kernel_spmd(
        nc, in_maps, core_ids=list(range(NCORES)), trace=trace)
    out = np.empty((B * S, D), np.float32)
    for i in range(NCORES):
        out[i * TOK:(i + 1) * TOK] = res.results[i]["outT"].T
    return out.reshape(B, S, D), res


def kernel(**inputs):
    out, _ = run(inputs)
    return out

